# revision 19
# baseline (speedup 1.0000x reference)
"""Trainium2 Bass kernel for nn_DihedralsPredictor (GNN message passing).

Strategy (8 NeuronCores, SPMD single NEFF):
  - Nodes are block-partitioned across cores; within a core, nodes are
    greedily permuted into 128-node destination windows balanced by degree.
    Edges live on the core owning their destination node, grouped per
    window, padded to a per-window tile count that is uniform across cores.
  - Per layer: the radial MLP runs feature-major on the TensorEngine in
    bf16 (the final softmax is saturated: bf16 end-to-end reproduces the
    fp32 reference exactly). The per-edge gather of h @ Wx uses GPSIMD
    dma_gather (4 SWDGE queues) from a DRAM table built by an AllGather of
    each core's node block. Scatter-sum to destinations is a one-hot
    matmul accumulated in PSUM; the self-connection tensor product
    accumulates into the same PSUM window.
  - Graph pooling is a per-window one-hot matmul, AllReduce across cores,
    and a small fp32 MLP head + softmax replicated on every core.
"""

import os
import sys

for _p in ("/opt/trn_rl_repo", "/root/.axon_site/_ro/trn_rl_repo"):
    if os.path.isdir(_p) and _p not in sys.path:
        sys.path.insert(0, _p)

import numpy as np
import ml_dtypes

import concourse.bass as bass
import concourse.bacc as bacc
import concourse.mybir as mybir
import concourse.tile as tile
from concourse import bass_utils

F32 = mybir.dt.float32
BF16 = mybir.dt.bfloat16
I16 = mybir.dt.int16
AF = mybir.ActivationFunctionType
OP = mybir.AluOpType

N_CORES = 8
P = 128          # partitions / window size
SUPER = 512      # edges per radial super-tile (4 tiles of 128)
CHUNK_TILES = 16  # gather chunk size in 128-edge tiles

_BUILD_CACHE = {}
_LAST_RESULT = None

if os.environ.get("KGNN_LDWOPT"):
    _orig_run_command = bass_utils.run_command

    def _run_command(argv, **kwargs):
        argv = ["--enable-ldw-opt=true" if a == "--enable-ldw-opt=false" else a
                for a in argv]
        return _orig_run_command(argv, **kwargs)

    bass_utils.run_command = _run_command
    import concourse.bass_utils as _bu
    _bu.run_command = _run_command


def _bf(x):
    return np.asarray(x, np.float32).astype(ml_dtypes.bfloat16)


# --------------------------------------------------------------------------
# Host-side preprocessing: shard, balance, sort, pad, pack.
# --------------------------------------------------------------------------
def _preprocess(inputs):
    x = np.asarray(inputs["x"], np.float32)
    node_attr = np.asarray(inputs["node_attr"], np.float32)
    edge_attr = np.asarray(inputs["edge_attr"], np.float32)
    ele = np.asarray(inputs["edge_length_embedding"], np.float32)
    edge_src = np.asarray(inputs["edge_src"], np.int64)
    edge_dst = np.asarray(inputs["edge_dst"], np.int64)
    batch = np.asarray(inputs["batch"], np.int64)

    NN, D = x.shape
    NA = node_attr.shape[1]
    SH = edge_attr.shape[1]
    NB = ele.shape[1]
    L = inputs["W0"].shape[0]
    NG = int(np.max(batch)) + 1
    NDIH = inputs["out_w"].shape[1]
    assert D == P and NB + SH <= 32

    npc = -(-NN // N_CORES)              # real nodes per core
    npcp = -(-npc // P) * P              # padded
    NW = npcp // P                       # windows per core
    NNP = npcp * N_CORES                 # padded global nodes

    core_of = edge_dst // npc
    loc_dst_orig = edge_dst - core_of * npc

    # --- balance windows: permute local node ids so window loads even out
    deg = np.zeros((N_CORES, npc), np.int64)
    np.add.at(deg, (core_of, loc_dst_orig), 1)
    newloc_tab = np.zeros((N_CORES, npc), np.int64)
    maxload = 0
    for c in range(N_CORES):
        order = np.argsort(-deg[c], kind="stable")
        loads = np.zeros(NW, np.int64)
        counts = np.zeros(NW, np.int64)
        for n in order:
            wsel = np.where(counts < P, loads, np.iinfo(np.int64).max)
            w = int(np.argmin(wsel))
            newloc_tab[c, n] = w * P + counts[w]
            counts[w] += 1
            loads[w] += deg[c, n]
        maxload = max(maxload, int(loads.max()))

    T_w = max(2, -(-maxload // P))
    tw = [T_w] * NW
    NT = T_w * NW
    padt = (-NT) % 4
    tw[-1] += padt
    NT += padt
    E_pad = NT * P
    S = E_pad // SUPER

    tile_start = np.zeros(NW + 1, np.int64)
    tile_start[1:] = np.cumsum(tw)
    w_of_tile = np.zeros(NT, np.int64)
    for w in range(NW):
        w_of_tile[tile_start[w]:tile_start[w + 1]] = w

    # gather chunks: CHUNK_TILES tiles each (aligned to supertiles)
    chunks = []
    t = 0
    while t < NT:
        n = min(CHUNK_TILES, NT - t)
        chunks.append((t, n))
        t += n
    NCH = len(chunks)
    COLS = SUPER * (-(-S // 4))

    loc_dst = newloc_tab[core_of, loc_dst_orig]
    win_of = loc_dst // P
    src_core = edge_src // npc
    src_gid = src_core * npcp + newloc_tab[src_core, edge_src % npc]

    per_core = []
    for c in range(N_CORES):
        sel = np.nonzero(core_of == c)[0]
        order = np.lexsort((edge_src[sel], loc_dst[sel]))
        sel = sel[order]
        wsel = win_of[sel]
        slots = np.zeros(len(sel), np.int64)
        pos = 0
        for w in range(NW):
            n_w = int((wsel == w).sum())
            assert n_w <= tw[w] * P, (c, w, n_w)
            slots[pos:pos + n_w] = tile_start[w] * P + np.arange(n_w)
            pos += n_w
        assert pos == len(sel)

        e_src = np.zeros(E_pad, np.int64)
        e_dst_rel = np.full(E_pad, -1.0, np.float32)
        e_feat = np.zeros((E_pad, 32), np.float32)
        e_src[slots] = src_gid[sel]
        e_dst_rel[slots] = (loc_dst[sel] - win_of[sel] * P).astype(np.float32)
        e_feat[slots, :NB] = ele[sel]
        e_feat[slots, NB:NB + SH] = edge_attr[sel]

        # gather index table [128, E_pad//16] int16 (16-partition wrap, x8)
        idx16 = np.zeros((P, E_pad // 16), np.int16)
        src16 = e_src.reshape(-1, 16).T.astype(np.int16)
        for k in range(8):
            idx16[16 * k:16 * (k + 1), :] = src16

        dstrel = e_dst_rel.reshape(NT, P).T.copy()

        feat = np.zeros((P, COLS), np.float32)
        ef = e_feat.reshape(S, SUPER, 32)
        for s in range(S):
            g, q = s % 4, s // 4
            feat[32 * g:32 * g + 32, SUPER * q:SUPER * (q + 1)] = ef[s].T

        # node-side arrays in permuted order
        n0, n1 = c * npc, min((c + 1) * npc, NN)
        nreal = n1 - n0
        xs = np.zeros((npcp, D), np.float32)
        at = np.zeros((npcp, NA), np.float32)
        po = np.zeros((npcp, NG), np.float32)
        nl = newloc_tab[c, :nreal]
        xs[nl] = x[n0:n1]
        at[nl] = node_attr[n0:n1]
        po[nl, batch[n0:n1]] = 1.0
        xsh = xs.reshape(NW, P, D).transpose(1, 0, 2).reshape(P, NW * D)
        xT = xs.reshape(NW, P, D).transpose(2, 0, 1).reshape(D, NW * P)
        attr = at.reshape(NW, P, NA).transpose(1, 0, 2).reshape(P, NW * NA)
        pooloh = po.reshape(NW, P, NG).transpose(1, 0, 2).reshape(P, NW * NG)

        per_core.append({
            "feat": np.ascontiguousarray(_bf(feat)),
            "idx": np.ascontiguousarray(idx16),
            "dstrel": np.ascontiguousarray(dstrel),
            "xsh": np.ascontiguousarray(_bf(xsh)),
            "xT": np.ascontiguousarray(_bf(xT)),
            "attr": np.ascontiguousarray(attr),
            "pooloh": np.ascontiguousarray(_bf(pooloh)),
        })

    # ---- shared weights ----
    W0 = np.asarray(inputs["W0"], np.float32)
    W1 = np.asarray(inputs["W1"], np.float32)
    W2 = np.asarray(inputs["W2"], np.float32)
    Wp = np.asarray(inputs["Wp"], np.float32)
    Wx = np.asarray(inputs["Wx"], np.float32)
    Wa = np.asarray(inputs["Wa"], np.float32)
    Wsc = np.asarray(inputs["Wsc"], np.float32)
    inv_z = np.float32(1.0 / np.sqrt(16.0))

    wmats = []
    widx = {}

    def addw(name, m):
        assert m.shape == (P, P), (name, m.shape)
        widx[name] = len(wmats)
        wmats.append(m.astype(np.float32))

    D2 = W1.shape[2]
    KC = D2 // P
    for l in range(L):
        w0r = np.zeros((P, P), np.float32)
        war = np.zeros((P, P), np.float32)
        for b in range(4):
            w0r[32 * b:32 * b + NB, :] = W0[l]
            war[32 * b + NB:32 * b + NB + SH, :] = Wa[l]
        addw(f"W0r{l}", w0r)
        addw(f"War{l}", war)
        for cdx in range(KC):
            addw(f"W1_{l}_{cdx}", W1[l][:, P * cdx:P * (cdx + 1)])
        w2p = (W2[l] @ Wp[l]) * inv_z
        for cdx in range(KC):
            addw(f"W2p_{l}_{cdx}", w2p[P * cdx:P * (cdx + 1), :])
        addw(f"Wx{l}", Wx[l])
        for a in range(NA):
            addw(f"Wsc_{l}_{a}", Wsc[l, :, a, :])
    addw("ident", np.eye(P, dtype=np.float32))
    iota = np.tile(np.arange(P, dtype=np.float32)[None, :], (P, 1))
    addw("iota", iota)
    wm = _bf(np.stack(wmats))

    shared = {
        "wm": np.ascontiguousarray(wm),
        "fc1w": np.asarray(inputs["fc1_w"], np.float32),
        "fc2w": np.asarray(inputs["fc2_w"], np.float32),
        "outw": np.asarray(inputs["out_w"], np.float32),
        "b1": np.ascontiguousarray(
            np.tile(np.asarray(inputs["fc1_b"], np.float32)[None, :], (NG, 1))),
        "b2": np.ascontiguousarray(
            np.tile(np.asarray(inputs["fc2_b"], np.float32)[None, :], (NG, 1))),
        "bo": np.ascontiguousarray(
            np.tile(np.asarray(inputs["out_b"], np.float32)[None, :], (NG, 1))),
        "identf": np.eye(P, dtype=np.float32),
    }

    cfg = dict(NN=NN, D=D, NA=NA, SH=SH, NB=NB, L=L, NG=NG, NDIH=NDIH,
               npc=npc, npcp=npcp, NW=NW, NNP=NNP, E_pad=E_pad,
               NT=NT, S=S, NCH=NCH, COLS=COLS,
               KC=KC, NM=len(wmats),
               H1=inputs["fc1_w"].shape[1], H2=inputs["fc2_w"].shape[1])
    aux = dict(widx=widx, tw=tuple(tw), chunks=tuple(chunks),
               w_of_tile=tuple(int(v) for v in w_of_tile),
               tile_start=tuple(int(v) for v in tile_start))
    return cfg, aux, per_core, shared


# --------------------------------------------------------------------------
# Device program
# --------------------------------------------------------------------------
def _build(cfg, aux):
    NW, S, NT = cfg["NW"], cfg["S"], cfg["NT"]
    NCH, COLS = cfg["NCH"], cfg["COLS"]
    L, NA, NG, NDIH = cfg["L"], cfg["NA"], cfg["NG"], cfg["NDIH"]
    KC, NM = cfg["KC"], cfg["NM"]
    NNP, npcp, E_pad = cfg["NNP"], cfg["npcp"], cfg["E_pad"]
    H1, H2 = cfg["H1"], cfg["H2"]
    D = cfg["D"]
    widx = aux["widx"]
    chunks = aux["chunks"]
    w_of_tile = aux["w_of_tile"]
    tile_start = aux["tile_start"]
    first_of = {tile_start[w]: w for w in range(NW)}
    last_of = {tile_start[w + 1] - 1: w for w in range(NW)}
    chunk_of_tile = {}
    for ci, (t0, n) in enumerate(chunks):
        for t in range(t0, t0 + n):
            chunk_of_tile[t] = (ci, t - t0)

    nc = bacc.Bacc("TRN2", target_bir_lowering=False, debug=False,
                   enable_asserts=False, num_devices=N_CORES,
                   num_swdge_queues=4)

    def din(name, shape, dt):
        return nc.dram_tensor(name, list(shape), dt, kind="ExternalInput").ap()

    feat_d = din("feat", [P, COLS], BF16)
    idx_d = din("idx", [P, E_pad // 16], I16)
    dstrel_d = din("dstrel", [P, NT], F32)
    xsh_d = din("xsh", [P, NW * D], BF16)
    xT_d = din("xT", [D, NW * P], BF16)
    attr_d = din("attr", [P, NW * NA], F32)
    pooloh_d = din("pooloh", [P, NW * NG], BF16)
    wm_d = din("wm", [NM, P, P], BF16)
    fc1w_d = din("fc1w", [D, H1], F32)
    fc2w_d = din("fc2w", [H1, H2], F32)
    outw_d = din("outw", [H2, NDIH], F32)
    b1_d = din("b1", [NG, H1], F32)
    b2_d = din("b2", [NG, H2], F32)
    bo_d = din("bo", [NG, NDIH], F32)
    identf_d = din("identf", [P, P], F32)
    out_d = nc.dram_tensor("out", [NG, NDIH], F32, kind="ExternalOutput").ap()

    with tile.TileContext(nc) as tc:
        with tc.tile_pool(name="res", bufs=1) as res, \
             tc.tile_pool(name="sb", bufs=2) as sb, \
             tc.tile_pool(name="sb8", bufs=16) as sb8, \
             tc.tile_pool(name="hxgp", bufs=6) as hxgp, \
             tc.tile_pool(name="ps_mmA", bufs=2, space="PSUM") as ps_mmA, \
             tc.tile_pool(name="ps_a2", bufs=2, space="PSUM") as ps_a2, \
             tc.tile_pool(name="ps_we", bufs=1, space="PSUM") as ps_we, \
             tc.tile_pool(name="ps_pT", bufs=1, space="PSUM") as ps_pT, \
             tc.tile_pool(name="ps_agg", bufs=1, space="PSUM") as ps_agg, \
             tc.tile_pool(name="ps_w", bufs=1, space="PSUM") as ps_w, \
             tc.tile_pool(name="dram", bufs=2, space="DRAM") as dram, \
             tc.tile_pool(name="dram1", bufs=1, space="DRAM") as dram1, \
             tc.tile_pool(name="hp", bufs=2) as h_pool:

            # ---- resident loads ----
            feat = res.tile([P, COLS], BF16, tag="feat")
            nc.sync.dma_start(feat[:], feat_d[:])
            idx = res.tile([P, E_pad // 16], I16, tag="idx")
            nc.sync.dma_start(idx[:], idx_d[:])
            dstrel = res.tile([P, NT], F32, tag="dstrel")
            nc.sync.dma_start(dstrel[:], dstrel_d[:])
            attr = res.tile([P, NW * NA], F32, tag="attr")
            nc.sync.dma_start(attr[:], attr_d[:])
            pooloh = res.tile([P, NW * NG], BF16, tag="pooloh")
            nc.sync.dma_start(pooloh[:], pooloh_d[:])
            wsb = res.tile([P, NM * P], BF16, tag="wsb")
            nc.sync.dma_start(wsb[:].rearrange("p (n f) -> p n f", n=NM),
                              wm_d[:].rearrange("n p f -> p n f"))

            def W(name):
                i = widx[name]
                return wsb[:, P * i:P * (i + 1)]

            ident = W("ident")
            iota = W("iota")

            h_tiles = []
            h0 = h_pool.tile([P, NW * D], BF16, tag="h")
            nc.sync.dma_start(h0[:], xsh_d[:])
            h_tiles.append(h0)

            g_sb = res.tile([NG, D], F32, tag="gsb")
            nc.vector.memset(g_sb[:], 0.0)

            bounce = [dram.tile([npcp, D], BF16, tag="bounce",
                                name=f"bounce{_l}") for _l in range(L)]
            table = [dram.tile([NNP, D], BF16, tag="table",
                               addr_space="Shared", name=f"table{_l}")
                     for _l in range(L)]

            def hx_window(l, h_t, w):
                hT_ps = ps_w.tile([P, P], BF16, tag="wmisc")
                nc.tensor.matmul(hT_ps[:], h_t[:, P * w:P * (w + 1)], ident,
                                 is_transpose=True, start=True, stop=True,
                                 skip_group_check=True)
                hT = sb.tile([P, P], BF16, tag="hT")
                nc.vector.tensor_copy(hT[:], hT_ps[:])
                hx_ps = ps_w.tile([P, P], F32, tag="wmisc")
                nc.tensor.matmul(hx_ps[:], hT[:], W(f"Wx{l}"),
                                 start=True, stop=True, skip_group_check=True)
                hx_sb = sb.tile([P, P], BF16, tag="hxsb")
                nc.vector.tensor_copy(hx_sb[:], hx_ps[:])
                nc.sync.dma_start(bounce[l][P * w:P * (w + 1), :], hx_sb[:])

            def allgather(l):
                nc.gpsimd.collective_compute(
                    "AllGather", OP.bypass,
                    ins=[bounce[l].opt()], outs=[table[l].opt()],
                    replica_groups=[list(range(N_CORES))])

            xT = res.tile([D, NW * P], BF16, tag="xT")
            nc.sync.dma_start(xT[:], xT_d[:])
            for w in range(NW):
                hx0_ps = ps_w.tile([P, P], F32, tag="wmisc")
                nc.tensor.matmul(hx0_ps[:], xT[:, P * w:P * (w + 1)],
                                 W("Wx0"), start=True, stop=True,
                                 skip_group_check=True)
                hx0_sb = sb.tile([P, P], BF16, tag="hxsb")
                nc.vector.tensor_copy(hx0_sb[:], hx0_ps[:])
                nc.sync.dma_start(bounce[0][P * w:P * (w + 1), :], hx0_sb[:])
            allgather(0)

            # ---- layers ----
            for l in range(L):
                h_cur = h_tiles[l]
                h_nxt = h_pool.tile([P, NW * D], BF16, tag="h")
                h_tiles.append(h_nxt)

                hxg = []
                for ci, (t0c, ntc) in enumerate(chunks):
                    gt = hxgp.tile([P, CHUNK_TILES, P], BF16, tag="hxg")
                    nc.gpsimd.dma_gather(
                        gt[:, :ntc, :], table[l][:],
                        idx[:, t0c * 8:(t0c + ntc) * 8],
                        num_idxs=ntc * P, num_idxs_reg=ntc * P, elem_size=D,
                        single_packet=False, queue_num=ci % 4)
                    hxg.append(gt)

                agg_ps = None
                for s in range(S):
                    g4, q = s % 4, s // 4
                    rhs = feat[32 * g4:32 * g4 + 32, SUPER * q:SUPER * (q + 1)]
                    tp = (32 * g4, 0)
                    a1_ps = ps_mmA.tile([P, SUPER], F32, tag="mmA")
                    nc.tensor.matmul(a1_ps[:],
                                     W(f"W0r{l}")[32 * g4:32 * g4 + 32, :],
                                     rhs, tile_position=tp,
                                     start=True, stop=True,
                                     skip_group_check=True)
                    ea_ps = ps_mmA.tile([P, SUPER], F32, tag="mmA")
                    nc.tensor.matmul(ea_ps[:],
                                     W(f"War{l}")[32 * g4:32 * g4 + 32, :],
                                     rhs, tile_position=tp,
                                     start=True, stop=True,
                                     skip_group_check=True)
                    a1 = sb.tile([P, SUPER], BF16, tag="a1")
                    nc.scalar.activation(a1[:], a1_ps[:], AF.Gelu_apprx_tanh)
                    a2_ps = ps_a2.tile([P, KC * SUPER], F32, tag="a2",
                                       bufs=1)
                    for cdx in range(KC):
                        nc.tensor.matmul(
                            a2_ps[:, SUPER * cdx:SUPER * (cdx + 1)],
                            W(f"W1_{l}_{cdx}"), a1[:],
                            start=True, stop=True, skip_group_check=True)
                    a2 = sb.tile([P, KC * SUPER], BF16, tag="a2s")
                    nc.scalar.activation(a2[:], a2_ps[:], AF.Gelu_apprx_tanh)
                    we_ps = ps_we.tile([P, SUPER], F32, tag="we")
                    for cdx in range(KC):
                        nc.tensor.matmul(we_ps[:], W(f"W2p_{l}_{cdx}"),
                                         a2[:, SUPER * cdx:SUPER * (cdx + 1)],
                                         start=(cdx == 0),
                                         stop=(cdx == KC - 1),
                                         skip_group_check=True)
                    ea_sb = sb.tile([P, SUPER], BF16, tag="ea_sb")
                    nc.vector.tensor_copy(ea_sb[:], ea_ps[:])
                    prod = sb8.tile([P, SUPER], BF16, tag="prod")
                    nc.vector.tensor_tensor(prod[:], we_ps[:], ea_sb[:],
                                            op=OP.mult)

                    TPS = SUPER // P
                    t0 = TPS * s
                    pT_ps = ps_pT.tile([P, SUPER], BF16, tag="pT")
                    for j in range(TPS):
                        nc.tensor.matmul(pT_ps[:, P * j:P * (j + 1)],
                                         prod[:, P * j:P * (j + 1)],
                                         ident, is_transpose=True,
                                         start=True, stop=True,
                                         skip_group_check=True)
                    ch0, g0 = chunk_of_tile[t0]
                    msg = sb.tile([P, SUPER], BF16, tag="msg", bufs=3)
                    nc.vector.tensor_tensor(
                        msg[:].rearrange("p (g e) -> p g e", g=TPS),
                        pT_ps[:].rearrange("p (g e) -> p g e", g=TPS),
                        hxg[ch0][:, g0:g0 + TPS, :], op=OP.mult)
                    oh = sb.tile([P, SUPER], BF16, tag="oh", bufs=3)
                    nc.vector.tensor_tensor(
                        oh[:].rearrange("p (g e) -> p g e", g=TPS),
                        iota.unsqueeze(1).to_broadcast((P, TPS, P)),
                        dstrel[:, t0:t0 + TPS].to_broadcast((P, TPS, P)),
                        op=OP.is_equal)
                    for j in range(TPS):
                        t = t0 + j
                        w = w_of_tile[t]
                        if t in first_of:
                            agg_ps = ps_agg.tile([P, P], F32, tag="agg")
                            for a in range(NA):
                                sch = sb.tile([P, P], BF16, tag="sch", bufs=4)
                                nc.scalar.mul(
                                    sch[:], h_cur[:, P * w:P * (w + 1)],
                                    attr[:, NA * w + a:NA * w + a + 1])
                                schT_ps = ps_w.tile([P, P], BF16, tag="wmisc")
                                nc.tensor.matmul(schT_ps[:], sch[:], ident,
                                                 is_transpose=True,
                                                 start=True, stop=True,
                                                 skip_group_check=True)
                                schT = sb.tile([P, P], BF16, tag="schT",
                                               bufs=4)
                                nc.vector.tensor_copy(schT[:], schT_ps[:])
                                nc.tensor.matmul(agg_ps[:], schT[:],
                                                 W(f"Wsc_{l}_{a}"),
                                                 start=(a == 0), stop=False,
                                                 skip_group_check=True)
                        nc.tensor.matmul(agg_ps[:], oh[:, P * j:P * (j + 1)],
                                         msg[:, P * j:P * (j + 1)],
                                         start=False, stop=(t in last_of),
                                         skip_group_check=True)
                        if t in last_of:
                            func = AF.Gelu_apprx_tanh if l < L - 1 else AF.Copy
                            nc.scalar.activation(
                                h_nxt[:, P * w:P * (w + 1)], agg_ps[:], func)
                            if l < L - 1:
                                hx_window(l + 1, h_nxt, w)
                            else:
                                pool_ps = ps_w.tile([NG, P], F32, tag="wmisc")
                                nc.tensor.matmul(
                                    pool_ps[:],
                                    pooloh[:, NG * w:NG * (w + 1)],
                                    h_nxt[:, P * w:P * (w + 1)],
                                    start=True, stop=True,
                                    skip_group_check=True)
                                nc.vector.tensor_tensor(g_sb[:], g_sb[:],
                                                        pool_ps[:], op=OP.add)
                if l < L - 1:
                    allgather(l + 1)

            # ---- AllReduce pooled g, then the MLP head (fp32) ----
            ar_in = dram1.tile([NG, D], F32, tag="arin")
            ar_out = dram1.tile([NG, D], F32, tag="arout", addr_space="Shared")
            nc.sync.dma_start(ar_in[:], g_sb[:])
            nc.gpsimd.collective_compute(
                "AllReduce", OP.add, ins=[ar_in.opt()], outs=[ar_out.opt()],
                replica_groups=[list(range(N_CORES))])
            g_full = res.tile([NG, D], F32, tag="gfull")
            nc.sync.dma_start(g_full[:], ar_out[:])

            identf = res.tile([P, P], F32, tag="identf")
            nc.sync.dma_start(identf[:], identf_d[:])
            fc1w = res.tile([D, H1], F32, tag="fc1w")
            nc.sync.dma_start(fc1w[:], fc1w_d[:])
            fc2w = res.tile([P, (H1 // P) * H2], F32, tag="fc2w")
            nc.sync.dma_start(
                fc2w[:].rearrange("p (c h) -> p c h", c=H1 // P),
                fc2w_d[:].rearrange("(c p) h -> p c h", p=P))
            outw = res.tile([P, (H2 // P) * NDIH], F32, tag="outw")
            nc.sync.dma_start(
                outw[:].rearrange("p (c h) -> p c h", c=H2 // P),
                outw_d[:].rearrange("(c p) h -> p c h", p=P))
            b1 = res.tile([NG, H1], F32, tag="b1")
            nc.sync.dma_start(b1[:], b1_d[:])
            b2 = res.tile([NG, H2], F32, tag="b2")
            nc.sync.dma_start(b2[:], b2_d[:])
            bo = res.tile([NG, NDIH], F32, tag="bo")
            nc.sync.dma_start(bo[:], bo_d[:])

            def transpose_cols(src, n_rows, tag):
                outs = []
                for cdx in range(n_rows // P):
                    t_ps = ps_w.tile([P, NG], F32, tag="wmisc")
                    nc.tensor.matmul(t_ps[:], src[:, P * cdx:P * (cdx + 1)],
                                     identf[:NG, :NG], is_transpose=True,
                                     start=True, stop=True,
                                     skip_group_check=True)
                    t_sb = sb.tile([P, NG], F32, tag=tag)
                    nc.vector.tensor_copy(t_sb[:], t_ps[:])
                    outs.append(t_sb)
                return outs

            gT = transpose_cols(g_full, D, "gT")
            f1_ps = ps_mmA.tile([NG, H1], F32, tag="mmA")
            nc.tensor.matmul(f1_ps[:], gT[0][:], fc1w[:], start=True,
                             stop=True, skip_group_check=True)
            t1 = sb.tile([NG, H1], F32, tag="t1")
            nc.vector.tensor_tensor(t1[:], f1_ps[:], b1[:], op=OP.add)
            t1g = sb.tile([NG, H1], F32, tag="t1g")
            nc.scalar.activation(t1g[:], t1[:], AF.Gelu_apprx_tanh)
            t1T = transpose_cols(t1g, H1, "t1T")
            f2_ps = ps_mmA.tile([NG, H2], F32, tag="mmA")
            for cdx in range(H1 // P):
                nc.tensor.matmul(f2_ps[:], t1T[cdx][:],
                                 fc2w[:, H2 * cdx:H2 * (cdx + 1)],
                                 start=(cdx == 0), stop=(cdx == H1 // P - 1),
                                 skip_group_check=True)
            t2 = sb.tile([NG, H2], F32, tag="t2")
            nc.vector.tensor_tensor(t2[:], f2_ps[:], b2[:], op=OP.add)
            t2g = sb.tile([NG, H2], F32, tag="t2g")
            nc.scalar.activation(t2g[:], t2[:], AF.Gelu_apprx_tanh)
            t2T = transpose_cols(t2g, H2, "t2T")
            lo_ps = ps_mmA.tile([NG, NDIH], F32, tag="mmA")
            for cdx in range(H2 // P):
                nc.tensor.matmul(lo_ps[:], t2T[cdx][:],
                                 outw[:, NDIH * cdx:NDIH * (cdx + 1)],
                                 start=(cdx == 0), stop=(cdx == H2 // P - 1),
                                 skip_group_check=True)
            logits = sb.tile([NG, NDIH], F32, tag="logits")
            nc.vector.tensor_tensor(logits[:], lo_ps[:], bo[:], op=OP.add)
            mx = sb.tile([NG, 1], F32, tag="mx")
            nc.vector.reduce_max(mx[:], logits[:], axis=mybir.AxisListType.X)
            nmx = sb.tile([NG, 1], F32, tag="nmx")
            nc.vector.tensor_scalar(nmx[:], mx[:], -1.0, None, op0=OP.mult)
            ex = sb.tile([NG, NDIH], F32, tag="ex")
            nc.scalar.activation(ex[:], logits[:], AF.Exp, bias=nmx[:, 0:1])
            sm = sb.tile([NG, 1], F32, tag="sm")
            nc.vector.reduce_sum(sm[:], ex[:], axis=mybir.AxisListType.X)
            rs = sb.tile([NG, 1], F32, tag="rs")
            nc.vector.reciprocal(rs[:], sm[:])
            probs = sb.tile([NG, NDIH], F32, tag="probs")
            nc.vector.tensor_scalar(probs[:], ex[:], rs[:, 0:1], None,
                                    op0=OP.mult)
            nc.sync.dma_start(out_d[:], probs[:])

    nc.compile()
    return nc


def kernel(**inputs):
    global _LAST_RESULT
    cfg, aux, per_core, shared = _preprocess(inputs)
    key = (tuple(sorted((k, v) for k, v in cfg.items()
                        if isinstance(v, (int, str)))),
           aux["tw"], aux["chunks"])
    if key not in _BUILD_CACHE:
        _BUILD_CACHE[key] = _build(cfg, aux)
    nc = _BUILD_CACHE[key]

    in_maps = [dict(shared, **pc) for pc in per_core]
    trace = bool(os.environ.get("BASS_TRACE"))
    if trace:
        bass_utils.upload_artifacts = lambda d: str(d)
    res = bass_utils.run_bass_kernel_spmd(
        nc, in_maps, core_ids=list(range(N_CORES)), trace=trace)
    _LAST_RESULT = res
    return res.results[0]["out"]
